# revision 67
# baseline (speedup 1.0000x reference)
"""GwcVolume (group-wise correlation cost volume) Trainium2 Bass kernel.

Problem: left/right features (2, 320, 96, 192) fp32. For each disparity
d in [0, 48): cost[b,g,d,h,w] = mean_c( L[b, g*8+c, h, w] * R[b, g*8+c, h, w-d] )
masked to 0 for w < d.  Output (2, 40, 48, 96, 192) fp32.

Sharding: 40 groups split across 8 cores (5 groups = 40 channels per core).
Per-core inputs slice cleanly along the channel dim; no inter-core comms.

Per-core algorithm (v3):
  - SBUF layout: partitions = (c 8, hq 16), free = (b 2, hr 6, w 192);
    h = hq*6 + hr.  Inputs cast fp32 -> fp16 once (ScalarE), prefetched one
    group ahead.  R stored with a 48-elem zero guard before each w-row.
  - Products in fp16, sliced to w >= w0 = 4*(d//4) (the masked w < d region
    is host-zero-filled).  Per disparity quad: one fused VectorE tensor_mul
    covers di 0..2 via a raw 4-dim AP (stride-0 broadcast of L over the
    di axis, stride -1 shift of R); the di=3 product is split GpSimd
    (hr 0:5) / VectorE (hr 5:6).  The 38:10 row ratio matches the engines'
    model rates, so DVE / GpSimd / PE all run ~88% busy.
  - Group-mean on TensorE: constant block-identity weights [128, 32]
    (wm[(c,hq), s*16+hq'] = 1/8 * delta[hq,hq']), col-tiled 4-wide
    (tile_position=(0, 32*di)) so 4 disparities share one PSUM tile.
    One matmul per (d, hr); PSUM rows padded to 256 f32 so no output
    crosses a 2KB PSUM bank.
  - ScalarE copies PSUM -> SBUF casting to fp16 into a 4-dq-wide staging
    tile; one DMA per (g, dq//4, di, b) ships 4 disparities (step-4 slice
    of the d axis).  The first/last quads are split finer (per-b, per
    hr-pair) to shorten pipeline fill/drain.  Host upcasts to fp32 and
    zero-fills the masked triangle.
"""

import numpy as np

B = 2
C = 320
H = 96
W = 192
GROUP = 40
MAX_DISP = 48
N_CORES = 8
G_PER = GROUP // N_CORES      # 5 groups per core
CPG = C // GROUP              # 8 channels per group
CC = G_PER * CPG              # 40 channels per core
HQ = 16                       # h = hq*HR + hr
HR = 6
FD = HR * W                   # 1152 free elements per partition
GUARD = 48
PITCH = 256                   # psum row pitch (f32): rows never cross banks

# GpSimd takes hr 0:POOL_HR of the di=3 product in every quad; VectorE takes
# the rest.  5/6 makes per-quad times match: DVE (36+2)*0.521 ~ Pool 10*1.983
# ~ PE 48*0.417 (in units of (192-w0) elements).
POOL_HR = 5

_cache = {}


def _w0(g, dq):
    # skip the masked w < 4*dq region everywhere: engines only ever read
    # regions written at the same offset, and the DMA may ship stale or
    # never-written bytes there -- the host zero-fills the whole triangle
    return 4 * dq


def _build_program():
    import concourse.bacc as bacc
    import concourse.tile as tile
    from concourse import mybir

    f32 = mybir.dt.float32
    f16 = mybir.dt.float16

    from concourse.ap import AP

    nc = bacc.Bacc("TRN2", target_bir_lowering=False, num_devices=N_CORES)
    # per-(b,g) channel block (8 ch x 96 x 192) is contiguous = [128, 1152]
    # with partitions=(c, hq), free=(hr, w); declare pre-reshaped for 2D DMAs
    left = nc.declare_dram_parameter("left", [B, G_PER, 128, FD], f32, isOutput=False)
    right = nc.declare_dram_parameter("right", [B, G_PER, 128, FD], f32, isOutput=False)
    wm = nc.declare_dram_parameter("wm", [128, 32], f16, isOutput=False)
    out = nc.declare_dram_parameter(
        "out", [B, G_PER, MAX_DISP, H, W], f16, isOutput=True
    )

    with tile.TileContext(nc) as tc:
        with (
            tc.tile_pool(name="singles", bufs=1) as singles,
            tc.tile_pool(name="stage", bufs=8) as stagep,
            tc.tile_pool(name="res", bufs=1) as res,
            tc.tile_pool(name="prod", bufs=4) as prodp,
            tc.tile_pool(name="oq", bufs=2) as oqp,
            tc.tile_pool(name="psum", bufs=2, space="PSUM") as psump,
        ):
            wm_s = singles.tile([128, 32], f16)
            nc.gpsimd.dma_start(out=wm_s[:, :], in_=wm[:, :])

            Lt, Rt = {}, {}
            for g in range(G_PER):
                Lg = res.tile([128, B, HR, W], f16, tag=f"L{g}")
                Rg = res.tile([128, B, HR, GUARD + W], f16, tag=f"R{g}")
                # only the last 3 guard columns are ever read (products
                # at w in [w0, d) look back at most d - w0 <= 3 elements, and
                # the host zero-fills the masked output region anyway)
                nc.gpsimd.memset(Rg[:, :, :, GUARD - 3 : GUARD], 0.0)
                Lt[g], Rt[g] = Lg, Rg

            def load_one(g, b, which):
                st = stagep.tile([128, FD], f32, tag="stage")
                src = left if which == 0 else right
                if not (g == 0 and b == 0 and which == 1):
                    nc.sync.dma_start(out=st[:, :], in_=src[b, g, :, :])
                if which == 0:
                    nc.scalar.copy(
                        out=Lt[g][:, b, :, :],
                        in_=st[:, :].rearrange("p (hr w) -> p hr w", w=W),
                    )
                elif g == 0 and b == 0:
                    # the first R load+cast is the critical startup chain:
                    # split the DMA and the cast so products start sooner
                    nc.sync.dma_start(
                        out=st[:, 0 : 2 * W], in_=src[b, g, :, 0 : 2 * W]
                    )
                    nc.sync.dma_start(
                        out=st[:, 2 * W : FD], in_=src[b, g, :, 2 * W : FD]
                    )
                    for h0, h1 in ((0, 2), (2, 4), (4, HR)):
                        nc.scalar.copy(
                            out=Rt[g][:, b, h0:h1, GUARD : GUARD + W],
                            in_=st[:, :].rearrange("p (hr w) -> p hr w", w=W)[
                                :, h0:h1, :
                            ],
                        )
                else:
                    nc.scalar.copy(
                        out=Rt[g][:, b, :, GUARD : GUARD + W],
                        in_=st[:, :].rearrange("p (hr w) -> p hr w", w=W),
                    )

            for b in range(B):
                for which in range(2):
                    load_one(0, b, which)

            def fused_mult(P, Lg, Rg, nd, d0, w0, b=None, h0=0, h1=HR):
                # one DVE op for disparities d0..d0+nd-1: di-dim via raw AP
                # (stride 0 broadcast on L, stride -1 shift on R); (b,hr)
                # merge into one stride-192/240 dim to stay within 4 AP dims
                X = W - w0
                if b is None:
                    pv = P[:, 0, 0, 0, w0:W]
                    lv = Lg[:, 0, 0, w0:W]
                    bh = [4 * HR, B * HR]
                else:
                    pv = P[:, 0, b, h0, w0:W]
                    lv = Lg[:, b, h0, w0:W]
                    bh = [4 * HR, h1 - h0]
                out_ap = AP(
                    pv.tensor, pv.offset,
                    [list(pv.ap[0]), [B * HR * W, nd], [W, bh[1]], [1, X]],
                )
                l_ap = AP(
                    lv.tensor, lv.offset,
                    [list(lv.ap[0]), [0, nd], [W, bh[1]], [1, X]],
                )
                rv = Rg[:, 0 if b is None else b, h0 if b is not None else 0, 0:1]
                r_ap = AP(
                    rv.tensor, rv.offset + GUARD + w0 - d0,
                    [list(rv.ap[0]), [-1, nd], [GUARD + W, bh[1]], [1, X]],
                )
                nc.vector.tensor_mul(out_ap, l_ap, r_ap)

            for g in range(G_PER):
                Lg, Rg = Lt[g], Rt[g]
                oq = None
                for dq in range(MAX_DISP // 4):
                    w0 = _w0(g, dq)
                    d0 = 4 * dq
                    if dq % 4 == 0:
                        oq = oqp.tile([128, 4, B, HR, W], f16, tag="oq")
                    P = prodp.tile([128, 4, B, HR, W], f16, tag="P")
                    d3 = d0 + 3
                    if (g == 0 and dq < 2) or (
                        g == G_PER - 1 and dq in (0, MAX_DISP // 4 - 1)
                    ):
                        # split per-b at the ends of the pipeline (and per
                        # hr-pair for the first and last quads) so the first
                        # matmuls start sooner / the last ones finish sooner
                        lastq = g == G_PER - 1 and dq == MAX_DISP // 4 - 1
                        for b in range(B):
                            if (g == 0 and dq == 0) or lastq:
                                for h0 in range(0, HR, 2):
                                    fused_mult(
                                        P, Lg, Rg, 3, d0, w0, b=b, h0=h0, h1=h0 + 2
                                    )
                            else:
                                fused_mult(P, Lg, Rg, 3, d0, w0, b=b)
                            if lastq:
                                nc.vector.tensor_mul(
                                    P[:, 3, b, :, w0:W],
                                    Lg[:, b, :, w0:W],
                                    Rg[:, b, :, GUARD + w0 - d3 : GUARD + W - d3],
                                )
                                continue
                            nc.gpsimd.tensor_mul(
                                P[:, 3, b, 0:POOL_HR, w0:W],
                                Lg[:, b, 0:POOL_HR, w0:W],
                                Rg[:, b, 0:POOL_HR, GUARD + w0 - d3 : GUARD + W - d3],
                            )
                            nc.vector.tensor_mul(
                                P[:, 3, b, POOL_HR:HR, w0:W],
                                Lg[:, b, POOL_HR:HR, w0:W],
                                Rg[:, b, POOL_HR:HR, GUARD + w0 - d3 : GUARD + W - d3],
                            )
                    else:
                        fused_mult(P, Lg, Rg, 3, d0, w0)
                        nc.gpsimd.tensor_mul(
                            P[:, 3, :, 0:POOL_HR, w0:W],
                            Lg[:, :, 0:POOL_HR, w0:W],
                            Rg[:, :, 0:POOL_HR, GUARD + w0 - d3 : GUARD + W - d3],
                        )
                        nc.vector.tensor_mul(
                            P[:, 3, :, POOL_HR:HR, w0:W],
                            Lg[:, :, POOL_HR:HR, w0:W],
                            Rg[:, :, POOL_HR:HR, GUARD + w0 - d3 : GUARD + W - d3],
                        )
                    if g + 1 < G_PER and dq in (4, 7):
                        # prefetch next group's inputs mid-group, when the
                        # ScalarE has slack between copies
                        b = 0 if dq == 4 else 1
                        load_one(g + 1, b, 0)
                        load_one(g + 1, b, 1)
                    for b in range(B):
                        pq = psump.tile([128, HR, PITCH], f32, tag="pq")
                        for hr in range(HR):
                            for di in range(4):
                                nc.tensor.matmul(
                                    pq[32 * di : 32 * di + 32, hr, w0:W],
                                    wm_s[:, :],
                                    P[:, di, b, hr, w0:W],
                                    start=True,
                                    stop=True,
                                    tile_position=(0, 32 * di),
                                )
                        if g == G_PER - 1 and dq == MAX_DISP // 4 - 1 and b == 1:
                            # very last copy: split across DVE (idle by now)
                            # and ScalarE to shorten the drain tail
                            nc.vector.tensor_scalar_mul(
                                oq[:, dq % 4, b, 0:3, w0:W], pq[:, 0:3, w0:W], 1.0
                            )
                            nc.scalar.copy(
                                out=oq[:, dq % 4, b, 3:HR, w0:W],
                                in_=pq[:, 3:HR, w0:W],
                            )
                        else:
                            nc.scalar.copy(
                                out=oq[:, dq % 4, b, :, w0:W],
                                in_=pq[:, :, w0:W],
                            )
                    last_oq = g == G_PER - 1 and dq // 4 == 2
                    if last_oq:
                        # ship each dq as soon as both its copies are done
                        # (b merged; only dq 11's four DMAs remain in the tail)
                        for di in range(4):
                            d = 4 * dq + di
                            nc.sync.dma_start(
                                out=out[:, g, d, :, :].rearrange(
                                    "b (hq hr) w -> hq b hr w", hq=HQ
                                ),
                                in_=oq[32 * di : 32 * di + HQ, dq % 4, :, :, :],
                            )
                    elif dq % 4 == 3:
                        dqq = dq // 4
                        for b in range(B):
                            for di in range(4):
                                dd0 = 16 * dqq + di
                                nc.sync.dma_start(
                                    out=out[b, g, dd0 : dd0 + 13 : 4, :, :].rearrange(
                                        "dd (hq hr) w -> hq dd hr w", hq=HQ
                                    ),
                                    in_=oq[32 * di : 32 * di + HQ, :, b, :, :],
                                )
    nc.compile()
    return nc


def _make_wm():
    wm = np.zeros((128, 32), np.float16)
    for c in range(CPG):
        for hq in range(HQ):
            wm[c * HQ + hq, hq] = 1.0 / CPG
            wm[c * HQ + hq, 16 + hq] = 1.0 / CPG
    return wm


def _run(left_feature, right_feature, trace=False):
    from concourse.bass_utils import run_bass_kernel_spmd

    if "nc" not in _cache:
        _cache["nc"] = _build_program()
    nc = _cache["nc"]

    left_feature = np.ascontiguousarray(np.asarray(left_feature, dtype=np.float32))
    right_feature = np.ascontiguousarray(np.asarray(right_feature, dtype=np.float32))
    wm = _make_wm()

    in_maps = []
    for i in range(N_CORES):
        c0 = i * CC
        lf = np.ascontiguousarray(left_feature[:, c0 : c0 + CC]).reshape(
            B, G_PER, 128, FD
        )
        rf = np.ascontiguousarray(right_feature[:, c0 : c0 + CC]).reshape(
            B, G_PER, 128, FD
        )
        in_maps.append({"left": lf, "right": rf, "wm": wm})
    res = run_bass_kernel_spmd(nc, in_maps, list(range(N_CORES)), trace=trace)
    shards = [res.results[i]["out"] for i in range(N_CORES)]
    full = np.concatenate([np.asarray(s) for s in shards], axis=1).astype(np.float32)
    # zero-fill the masked triangle (w < d), which the kernel never computes
    for d in range(1, MAX_DISP):
        full[:, :, d, :, :d] = 0.0
    return full, res


def kernel(left_feature, right_feature):
    full, _ = _run(left_feature, right_feature, trace=False)
    return full


# revision 69
# speedup vs baseline: 1.0006x; 1.0006x over previous
"""GwcVolume (group-wise correlation cost volume) Trainium2 Bass kernel.

Problem: left/right features (2, 320, 96, 192) fp32. For each disparity
d in [0, 48): cost[b,g,d,h,w] = mean_c( L[b, g*8+c, h, w] * R[b, g*8+c, h, w-d] )
masked to 0 for w < d.  Output (2, 40, 48, 96, 192) fp32.

Sharding: 40 groups split across 8 cores (5 groups = 40 channels per core).
Per-core inputs slice cleanly along the channel dim; no inter-core comms.

Per-core algorithm (v3):
  - SBUF layout: partitions = (c 8, hq 16), free = (b 2, hr 6, w 192);
    h = hq*6 + hr.  Inputs cast fp32 -> fp16 once (ScalarE), prefetched one
    group ahead.  R stored with a 48-elem zero guard before each w-row.
  - Products in fp16, sliced to w >= w0 = 4*(d//4) (the masked w < d region
    is host-zero-filled).  Per disparity quad: one fused VectorE tensor_mul
    covers di 0..2 via a raw 4-dim AP (stride-0 broadcast of L over the
    di axis, stride -1 shift of R); the di=3 product is split GpSimd
    (hr 0:5) / VectorE (hr 5:6).  The 38:10 row ratio matches the engines'
    model rates, so DVE / GpSimd / PE all run ~88% busy.
  - Group-mean on TensorE: constant block-identity weights [128, 32]
    (wm[(c,hq), s*16+hq'] = 1/8 * delta[hq,hq']), col-tiled 4-wide
    (tile_position=(0, 32*di)) so 4 disparities share one PSUM tile.
    One matmul per (d, hr); PSUM rows padded to 256 f32 so no output
    crosses a 2KB PSUM bank.
  - ScalarE copies PSUM -> SBUF casting to fp16 into a 4-dq-wide staging
    tile; one DMA per (g, dq//4, di, b) ships 4 disparities (step-4 slice
    of the d axis).  The first/last quads are split finer (per-b, per
    hr-pair) to shorten pipeline fill/drain.  Host upcasts to fp32 and
    zero-fills the masked triangle.
"""

import numpy as np

B = 2
C = 320
H = 96
W = 192
GROUP = 40
MAX_DISP = 48
N_CORES = 8
G_PER = GROUP // N_CORES      # 5 groups per core
CPG = C // GROUP              # 8 channels per group
CC = G_PER * CPG              # 40 channels per core
HQ = 16                       # h = hq*HR + hr
HR = 6
FD = HR * W                   # 1152 free elements per partition
GUARD = 8
PITCH = 256                   # psum row pitch (f32): rows never cross banks

# GpSimd takes hr 0:POOL_HR of the di=3 product in every quad; VectorE takes
# the rest.  5/6 makes per-quad times match: DVE (36+2)*0.521 ~ Pool 10*1.983
# ~ PE 48*0.417 (in units of (192-w0) elements).
POOL_HR = 5

_cache = {}


def _w0(g, dq):
    # skip the masked w < 4*dq region everywhere: engines only ever read
    # regions written at the same offset, and the DMA may ship stale or
    # never-written bytes there -- the host zero-fills the whole triangle
    return 4 * dq


def _build_program():
    import concourse.bacc as bacc
    import concourse.tile as tile
    from concourse import mybir

    f32 = mybir.dt.float32
    f16 = mybir.dt.float16

    from concourse.ap import AP

    nc = bacc.Bacc("TRN2", target_bir_lowering=False, num_devices=N_CORES)
    # per-(b,g) channel block (8 ch x 96 x 192) is contiguous = [128, 1152]
    # with partitions=(c, hq), free=(hr, w); declare pre-reshaped for 2D DMAs
    left = nc.declare_dram_parameter("left", [B, G_PER, 128, FD], f32, isOutput=False)
    right = nc.declare_dram_parameter("right", [B, G_PER, 128, FD], f32, isOutput=False)
    wm = nc.declare_dram_parameter("wm", [128, 32], f16, isOutput=False)
    out = nc.declare_dram_parameter(
        "out", [B, G_PER, MAX_DISP, H, W], f16, isOutput=True
    )

    with tile.TileContext(nc) as tc:
        with (
            tc.tile_pool(name="singles", bufs=1) as singles,
            tc.tile_pool(name="stage", bufs=10) as stagep,
            tc.tile_pool(name="res", bufs=1) as res,
            tc.tile_pool(name="prod", bufs=4) as prodp,
            tc.tile_pool(name="oq", bufs=2) as oqp,
            tc.tile_pool(name="psum", bufs=2, space="PSUM") as psump,
        ):
            wm_s = singles.tile([128, 32], f16)
            nc.gpsimd.dma_start(out=wm_s[:, :], in_=wm[:, :])

            Lt, Rt = {}, {}
            for g in range(G_PER):
                Lg = res.tile([128, B, HR, W], f16, tag=f"L{g}")
                Rg = res.tile([128, B, HR, GUARD + W], f16, tag=f"R{g}")
                # only the last 3 guard columns are ever read (products
                # at w in [w0, d) look back at most d - w0 <= 3 elements, and
                # the host zero-fills the masked output region anyway)
                nc.gpsimd.memset(Rg[:, :, :, GUARD - 3 : GUARD], 0.0)
                Lt[g], Rt[g] = Lg, Rg

            def load_one(g, b, which):
                st = stagep.tile([128, FD], f32, tag="stage")
                src = left if which == 0 else right
                if not (g == 0 and b == 0 and which == 1):
                    nc.sync.dma_start(out=st[:, :], in_=src[b, g, :, :])
                if which == 0:
                    nc.scalar.copy(
                        out=Lt[g][:, b, :, :],
                        in_=st[:, :].rearrange("p (hr w) -> p hr w", w=W),
                    )
                elif g == 0 and b == 0:
                    # the first R load+cast is the critical startup chain:
                    # split the DMA and the cast so products start sooner
                    nc.sync.dma_start(
                        out=st[:, 0 : 2 * W], in_=src[b, g, :, 0 : 2 * W]
                    )
                    nc.sync.dma_start(
                        out=st[:, 2 * W : FD], in_=src[b, g, :, 2 * W : FD]
                    )
                    for h0, h1 in ((0, 2), (2, 4), (4, HR)):
                        nc.scalar.copy(
                            out=Rt[g][:, b, h0:h1, GUARD : GUARD + W],
                            in_=st[:, :].rearrange("p (hr w) -> p hr w", w=W)[
                                :, h0:h1, :
                            ],
                        )
                else:
                    nc.scalar.copy(
                        out=Rt[g][:, b, :, GUARD : GUARD + W],
                        in_=st[:, :].rearrange("p (hr w) -> p hr w", w=W),
                    )

            for b in range(B):
                for which in range(2):
                    load_one(0, b, which)

            def fused_mult(P, Lg, Rg, nd, d0, w0, b=None, h0=0, h1=HR):
                # one DVE op for disparities d0..d0+nd-1: di-dim via raw AP
                # (stride 0 broadcast on L, stride -1 shift on R); (b,hr)
                # merge into one stride-192/240 dim to stay within 4 AP dims
                X = W - w0
                if b is None:
                    pv = P[:, 0, 0, 0, w0:W]
                    lv = Lg[:, 0, 0, w0:W]
                    bh = [4 * HR, B * HR]
                else:
                    pv = P[:, 0, b, h0, w0:W]
                    lv = Lg[:, b, h0, w0:W]
                    bh = [4 * HR, h1 - h0]
                out_ap = AP(
                    pv.tensor, pv.offset,
                    [list(pv.ap[0]), [B * HR * W, nd], [W, bh[1]], [1, X]],
                )
                l_ap = AP(
                    lv.tensor, lv.offset,
                    [list(lv.ap[0]), [0, nd], [W, bh[1]], [1, X]],
                )
                rv = Rg[:, 0 if b is None else b, h0 if b is not None else 0, 0:1]
                r_ap = AP(
                    rv.tensor, rv.offset + GUARD + w0 - d0,
                    [list(rv.ap[0]), [-1, nd], [GUARD + W, bh[1]], [1, X]],
                )
                nc.vector.tensor_mul(out_ap, l_ap, r_ap)

            for g in range(G_PER):
                Lg, Rg = Lt[g], Rt[g]
                oq = None
                for dq in range(MAX_DISP // 4):
                    w0 = _w0(g, dq)
                    d0 = 4 * dq
                    if dq % 4 == 0:
                        oq = oqp.tile([128, 4, B, HR, W], f16, tag="oq")
                    P = prodp.tile([128, 4, B, HR, W], f16, tag="P")
                    d3 = d0 + 3
                    if (g == 0 and dq < 2) or (
                        g == G_PER - 1 and dq in (0, MAX_DISP // 4 - 1)
                    ):
                        # split per-b at the ends of the pipeline (and per
                        # hr-pair for the first and last quads) so the first
                        # matmuls start sooner / the last ones finish sooner
                        lastq = g == G_PER - 1 and dq == MAX_DISP // 4 - 1
                        for b in range(B):
                            if (g == 0 and dq == 0) or lastq:
                                for h0 in range(0, HR, 2):
                                    fused_mult(
                                        P, Lg, Rg, 3, d0, w0, b=b, h0=h0, h1=h0 + 2
                                    )
                            else:
                                fused_mult(P, Lg, Rg, 3, d0, w0, b=b)
                            if lastq:
                                nc.vector.tensor_mul(
                                    P[:, 3, b, :, w0:W],
                                    Lg[:, b, :, w0:W],
                                    Rg[:, b, :, GUARD + w0 - d3 : GUARD + W - d3],
                                )
                                continue
                            nc.gpsimd.tensor_mul(
                                P[:, 3, b, 0:POOL_HR, w0:W],
                                Lg[:, b, 0:POOL_HR, w0:W],
                                Rg[:, b, 0:POOL_HR, GUARD + w0 - d3 : GUARD + W - d3],
                            )
                            nc.vector.tensor_mul(
                                P[:, 3, b, POOL_HR:HR, w0:W],
                                Lg[:, b, POOL_HR:HR, w0:W],
                                Rg[:, b, POOL_HR:HR, GUARD + w0 - d3 : GUARD + W - d3],
                            )
                    else:
                        fused_mult(P, Lg, Rg, 3, d0, w0)
                        nc.gpsimd.tensor_mul(
                            P[:, 3, :, 0:POOL_HR, w0:W],
                            Lg[:, :, 0:POOL_HR, w0:W],
                            Rg[:, :, 0:POOL_HR, GUARD + w0 - d3 : GUARD + W - d3],
                        )
                        nc.vector.tensor_mul(
                            P[:, 3, :, POOL_HR:HR, w0:W],
                            Lg[:, :, POOL_HR:HR, w0:W],
                            Rg[:, :, POOL_HR:HR, GUARD + w0 - d3 : GUARD + W - d3],
                        )
                    # prefetch two groups ahead so the late group
                    # boundaries carry no cast traffic at all
                    if g == 0 and dq in (4, 7):
                        b = 0 if dq == 4 else 1
                        load_one(1, b, 0)
                        load_one(1, b, 1)
                    elif g == 0 and dq in (9, 11):
                        b = 0 if dq == 9 else 1
                        load_one(2, b, 0)
                        load_one(2, b, 1)
                    elif 1 <= g <= 2 and dq in (4, 7):
                        b = 0 if dq == 4 else 1
                        load_one(g + 2, b, 0)
                        load_one(g + 2, b, 1)
                    for b in range(B):
                        pq = psump.tile([128, HR, PITCH], f32, tag="pq")
                        for hr in range(HR):
                            for di in range(4):
                                nc.tensor.matmul(
                                    pq[32 * di : 32 * di + 32, hr, w0:W],
                                    wm_s[:, :],
                                    P[:, di, b, hr, w0:W],
                                    start=True,
                                    stop=True,
                                    tile_position=(0, 32 * di),
                                )
                        if g == G_PER - 1 and dq == MAX_DISP // 4 - 1 and b == 1:
                            # very last copy: split across DVE (idle by now)
                            # and ScalarE to shorten the drain tail
                            nc.vector.tensor_scalar_mul(
                                oq[:, dq % 4, b, 0:3, w0:W], pq[:, 0:3, w0:W], 1.0
                            )
                            nc.scalar.copy(
                                out=oq[:, dq % 4, b, 3:HR, w0:W],
                                in_=pq[:, 3:HR, w0:W],
                            )
                        else:
                            nc.scalar.copy(
                                out=oq[:, dq % 4, b, :, w0:W],
                                in_=pq[:, :, w0:W],
                            )
                    last_oq = g == G_PER - 1 and dq // 4 == 2
                    if last_oq:
                        # ship each dq as soon as both its copies are done
                        # (b merged; only dq 11's four DMAs remain in the tail)
                        for di in range(4):
                            d = 4 * dq + di
                            nc.sync.dma_start(
                                out=out[:, g, d, :, :].rearrange(
                                    "b (hq hr) w -> hq b hr w", hq=HQ
                                ),
                                in_=oq[32 * di : 32 * di + HQ, dq % 4, :, :, :],
                            )
                    elif dq % 4 == 3:
                        dqq = dq // 4
                        for b in range(B):
                            for di in range(4):
                                dd0 = 16 * dqq + di
                                nc.sync.dma_start(
                                    out=out[b, g, dd0 : dd0 + 13 : 4, :, :].rearrange(
                                        "dd (hq hr) w -> hq dd hr w", hq=HQ
                                    ),
                                    in_=oq[32 * di : 32 * di + HQ, :, b, :, :],
                                )
    nc.compile()
    return nc


def _make_wm():
    wm = np.zeros((128, 32), np.float16)
    for c in range(CPG):
        for hq in range(HQ):
            wm[c * HQ + hq, hq] = 1.0 / CPG
            wm[c * HQ + hq, 16 + hq] = 1.0 / CPG
    return wm


def _run(left_feature, right_feature, trace=False):
    from concourse.bass_utils import run_bass_kernel_spmd

    if "nc" not in _cache:
        _cache["nc"] = _build_program()
    nc = _cache["nc"]

    left_feature = np.ascontiguousarray(np.asarray(left_feature, dtype=np.float32))
    right_feature = np.ascontiguousarray(np.asarray(right_feature, dtype=np.float32))
    wm = _make_wm()

    in_maps = []
    for i in range(N_CORES):
        c0 = i * CC
        lf = np.ascontiguousarray(left_feature[:, c0 : c0 + CC]).reshape(
            B, G_PER, 128, FD
        )
        rf = np.ascontiguousarray(right_feature[:, c0 : c0 + CC]).reshape(
            B, G_PER, 128, FD
        )
        in_maps.append({"left": lf, "right": rf, "wm": wm})
    res = run_bass_kernel_spmd(nc, in_maps, list(range(N_CORES)), trace=trace)
    shards = [res.results[i]["out"] for i in range(N_CORES)]
    full = np.concatenate([np.asarray(s) for s in shards], axis=1).astype(np.float32)
    # zero-fill the masked triangle (w < d), which the kernel never computes
    for d in range(1, MAX_DISP):
        full[:, :, d, :, :d] = 0.0
    return full, res


def kernel(left_feature, right_feature):
    full, _ = _run(left_feature, right_feature, trace=False)
    return full


# revision 75
# speedup vs baseline: 1.0028x; 1.0022x over previous
"""GwcVolume (group-wise correlation cost volume) Trainium2 Bass kernel.

Problem: left/right features (2, 320, 96, 192) fp32. For each disparity
d in [0, 48): cost[b,g,d,h,w] = mean_c( L[b, g*8+c, h, w] * R[b, g*8+c, h, w-d] )
masked to 0 for w < d.  Output (2, 40, 48, 96, 192) fp32.

Sharding: 40 groups split across 8 cores (5 groups = 40 channels per core).
Per-core inputs slice cleanly along the channel dim; no inter-core comms.

Per-core algorithm (v3):
  - SBUF layout: partitions = (c 8, hq 16), free = (b 2, hr 6, w 192);
    h = hq*6 + hr.  Inputs cast fp32 -> fp16 once (ScalarE), prefetched one
    group ahead.  R stored with a 48-elem zero guard before each w-row.
  - Products in fp16, sliced to w >= w0 = 4*(d//4) (the masked w < d region
    is host-zero-filled).  Per disparity quad: one fused VectorE tensor_mul
    covers di 0..2 via a raw 4-dim AP (stride-0 broadcast of L over the
    di axis, stride -1 shift of R); the di=3 product is split GpSimd
    (hr 0:5) / VectorE (hr 5:6).  The 38:10 row ratio matches the engines'
    model rates, so DVE / GpSimd / PE all run ~88% busy.
  - Group-mean on TensorE: constant block-identity weights [128, 32]
    (wm[(c,hq), s*16+hq'] = 1/8 * delta[hq,hq']), col-tiled 4-wide
    (tile_position=(0, 32*di)) so 4 disparities share one PSUM tile.
    One matmul per (d, hr); PSUM rows padded to 256 f32 so no output
    crosses a 2KB PSUM bank.
  - ScalarE copies PSUM -> SBUF casting to fp16 into a 4-dq-wide staging
    tile; one DMA per (g, dq//4, di, b) ships 4 disparities (step-4 slice
    of the d axis).  The first/last quads are split finer (per-b, per
    hr-pair) to shorten pipeline fill/drain.  Host upcasts to fp32 and
    zero-fills the masked triangle.
"""

import numpy as np

B = 2
C = 320
H = 96
W = 192
GROUP = 40
MAX_DISP = 48
N_CORES = 8
G_PER = GROUP // N_CORES      # 5 groups per core
CPG = C // GROUP              # 8 channels per group
CC = G_PER * CPG              # 40 channels per core
HQ = 16                       # h = hq*HR + hr
HR = 6
FD = HR * W                   # 1152 free elements per partition
GUARD = 8
PITCH = 256                   # psum row pitch (f32): rows never cross banks

# GpSimd takes hr 0:POOL_HR of the di=3 product in every quad; VectorE takes
# the rest.  5/6 makes per-quad times match: DVE (36+2)*0.521 ~ Pool 10*1.983
# ~ PE 48*0.417 (in units of (192-w0) elements).
POOL_HR = 5

_cache = {}


def _w0(g, dq):
    # skip the masked w < 4*dq region everywhere: engines only ever read
    # regions written at the same offset, and the DMA may ship stale or
    # never-written bytes there -- the host zero-fills the whole triangle
    return 4 * dq


def _build_program():
    import concourse.bacc as bacc
    import concourse.tile as tile
    from concourse import mybir

    f32 = mybir.dt.float32
    f16 = mybir.dt.float16

    from concourse.ap import AP

    nc = bacc.Bacc("TRN2", target_bir_lowering=False, num_devices=N_CORES)
    # per-(b,g) channel block (8 ch x 96 x 192) is contiguous = [128, 1152]
    # with partitions=(c, hq), free=(hr, w); declare pre-reshaped for 2D DMAs
    left = nc.declare_dram_parameter("left", [B, G_PER, 128, FD], f32, isOutput=False)
    right = nc.declare_dram_parameter("right", [B, G_PER, 128, FD], f32, isOutput=False)
    wm = nc.declare_dram_parameter("wm", [128, 32], f16, isOutput=False)
    out = nc.declare_dram_parameter(
        "out", [B, G_PER, MAX_DISP, H, W], f16, isOutput=True
    )

    with tile.TileContext(nc) as tc:
        with (
            tc.tile_pool(name="singles", bufs=1) as singles,
            tc.tile_pool(name="stage", bufs=10) as stagep,
            tc.tile_pool(name="res", bufs=1) as res,
            tc.tile_pool(name="prod", bufs=4) as prodp,
            tc.tile_pool(name="oq", bufs=2) as oqp,
            tc.tile_pool(name="psum", bufs=2, space="PSUM") as psump,
        ):
            wm_s = singles.tile([128, 32], f16)
            nc.gpsimd.dma_start(out=wm_s[:, :], in_=wm[:, :])

            Lt, Rt = {}, {}
            for g in range(G_PER):
                Lg = res.tile([128, B, HR, W], f16, tag=f"L{g}")
                Rg = res.tile([128, B, HR, GUARD + W], f16, tag=f"R{g}")
                # only the last 3 guard columns are ever read (products
                # at w in [w0, d) look back at most d - w0 <= 3 elements, and
                # the host zero-fills the masked output region anyway)
                nc.gpsimd.memset(Rg[:, :, :, GUARD - 3 : GUARD], 0.0)
                Lt[g], Rt[g] = Lg, Rg

            def load_one(g, b, which):
                st = stagep.tile([128, FD], f32, tag="stage")
                src = left if which == 0 else right
                if not (g == 0 and b == 0 and which == 1):
                    nc.sync.dma_start(out=st[:, :], in_=src[b, g, :, :])
                if which == 0:
                    nc.scalar.copy(
                        out=Lt[g][:, b, :, :],
                        in_=st[:, :].rearrange("p (hr w) -> p hr w", w=W),
                    )
                elif g == 0 and b == 0:
                    # the first R load+cast is the critical startup chain:
                    # split the DMA and the cast so products start sooner
                    nc.sync.dma_start(
                        out=st[:, 0 : 2 * W], in_=src[b, g, :, 0 : 2 * W]
                    )
                    nc.sync.dma_start(
                        out=st[:, 2 * W : FD], in_=src[b, g, :, 2 * W : FD]
                    )
                    for h0, h1 in ((0, 2), (2, 4), (4, HR)):
                        nc.scalar.copy(
                            out=Rt[g][:, b, h0:h1, GUARD : GUARD + W],
                            in_=st[:, :].rearrange("p (hr w) -> p hr w", w=W)[
                                :, h0:h1, :
                            ],
                        )
                else:
                    nc.scalar.copy(
                        out=Rt[g][:, b, :, GUARD : GUARD + W],
                        in_=st[:, :].rearrange("p (hr w) -> p hr w", w=W),
                    )

            for b in range(B):
                for which in range(2):
                    load_one(0, b, which)

            def fused_mult(P, Lg, Rg, nd, d0, w0, b=None, h0=0, h1=HR):
                # one DVE op for disparities d0..d0+nd-1: di-dim via raw AP
                # (stride 0 broadcast on L, stride -1 shift on R); (b,hr)
                # merge into one stride-192/240 dim to stay within 4 AP dims
                X = W - w0
                if b is None:
                    pv = P[:, 0, 0, 0, w0:W]
                    lv = Lg[:, 0, 0, w0:W]
                    bh = [4 * HR, B * HR]
                else:
                    pv = P[:, 0, b, h0, w0:W]
                    lv = Lg[:, b, h0, w0:W]
                    bh = [4 * HR, h1 - h0]
                out_ap = AP(
                    pv.tensor, pv.offset,
                    [list(pv.ap[0]), [B * HR * W, nd], [W, bh[1]], [1, X]],
                )
                l_ap = AP(
                    lv.tensor, lv.offset,
                    [list(lv.ap[0]), [0, nd], [W, bh[1]], [1, X]],
                )
                rv = Rg[:, 0 if b is None else b, h0 if b is not None else 0, 0:1]
                r_ap = AP(
                    rv.tensor, rv.offset + GUARD + w0 - d0,
                    [list(rv.ap[0]), [-1, nd], [GUARD + W, bh[1]], [1, X]],
                )
                nc.vector.tensor_mul(out_ap, l_ap, r_ap)

            for g in range(G_PER):
                Lg, Rg = Lt[g], Rt[g]
                oq = None
                for dq in range(MAX_DISP // 4):
                    w0 = _w0(g, dq)
                    d0 = 4 * dq
                    if dq % 4 == 0:
                        oq = oqp.tile([128, 4, B, HR, W], f16, tag="oq")
                    P = prodp.tile([128, 4, B, HR, W], f16, tag="P")
                    d3 = d0 + 3
                    if (g == 0 and dq < 2) or (
                        g == G_PER - 1 and dq in (0, MAX_DISP // 4 - 1)
                    ):
                        # split per-b at the ends of the pipeline (and per
                        # hr-pair for the first and last quads) so the first
                        # matmuls start sooner / the last ones finish sooner
                        lastq = g == G_PER - 1 and dq == MAX_DISP // 4 - 1
                        for b in range(B):
                            if (g == 0 and dq == 0) or lastq:
                                for h0 in range(0, HR, 2):
                                    fused_mult(
                                        P, Lg, Rg, 3, d0, w0, b=b, h0=h0, h1=h0 + 2
                                    )
                            else:
                                fused_mult(P, Lg, Rg, 3, d0, w0, b=b)
                            if lastq:
                                nc.vector.tensor_mul(
                                    P[:, 3, b, :, w0:W],
                                    Lg[:, b, :, w0:W],
                                    Rg[:, b, :, GUARD + w0 - d3 : GUARD + W - d3],
                                )
                                continue
                            nc.gpsimd.tensor_mul(
                                P[:, 3, b, 0:POOL_HR, w0:W],
                                Lg[:, b, 0:POOL_HR, w0:W],
                                Rg[:, b, 0:POOL_HR, GUARD + w0 - d3 : GUARD + W - d3],
                            )
                            nc.vector.tensor_mul(
                                P[:, 3, b, POOL_HR:HR, w0:W],
                                Lg[:, b, POOL_HR:HR, w0:W],
                                Rg[:, b, POOL_HR:HR, GUARD + w0 - d3 : GUARD + W - d3],
                            )
                    else:
                        fused_mult(P, Lg, Rg, 3, d0, w0)
                        nc.gpsimd.tensor_mul(
                            P[:, 3, :, 0:POOL_HR, w0:W],
                            Lg[:, :, 0:POOL_HR, w0:W],
                            Rg[:, :, 0:POOL_HR, GUARD + w0 - d3 : GUARD + W - d3],
                        )
                        nc.vector.tensor_mul(
                            P[:, 3, :, POOL_HR:HR, w0:W],
                            Lg[:, :, POOL_HR:HR, w0:W],
                            Rg[:, :, POOL_HR:HR, GUARD + w0 - d3 : GUARD + W - d3],
                        )
                    # prefetch two groups ahead so the late group
                    # boundaries carry no cast traffic at all
                    if g == 0 and dq in (4, 7):
                        b = 0 if dq == 4 else 1
                        load_one(1, b, 0)
                        load_one(1, b, 1)
                    elif g == 0 and dq in (9, 11):
                        b = 0 if dq == 9 else 1
                        load_one(2, b, 0)
                        load_one(2, b, 1)
                    elif 1 <= g <= 2 and dq in (4, 7):
                        b = 0 if dq == 4 else 1
                        load_one(g + 2, b, 0)
                        load_one(g + 2, b, 1)
                    for b in range(B):
                        pq = psump.tile([128, HR, PITCH], f32, tag="pq")
                        for hr in range(HR):
                            for di in range(4):
                                nc.tensor.matmul(
                                    pq[32 * di : 32 * di + 32, hr, w0:W],
                                    wm_s[:, :],
                                    P[:, di, b, hr, w0:W],
                                    start=True,
                                    stop=True,
                                    tile_position=(0, 32 * di),
                                )
                        if g == G_PER - 1 and dq == MAX_DISP // 4 - 1:
                            # very last copy: split across DVE (idle by now)
                            # and ScalarE to shorten the drain tail
                            nc.vector.tensor_scalar_mul(
                                oq[:, dq % 4, b, :, w0:W], pq[:, :, w0:W], 1.0
                            )
                        else:
                            nc.scalar.copy(
                                out=oq[:, dq % 4, b, :, w0:W],
                                in_=pq[:, :, w0:W],
                            )
                    last_oq = g == G_PER - 1 and dq // 4 == 2
                    if last_oq:
                        # ship each dq as soon as both its copies are done
                        # (b merged; only dq 11's four DMAs remain in the tail)
                        for di in range(4):
                            d = 4 * dq + di
                            nc.sync.dma_start(
                                out=out[:, g, d, :, :].rearrange(
                                    "b (hq hr) w -> hq b hr w", hq=HQ
                                ),
                                in_=oq[32 * di : 32 * di + HQ, dq % 4, :, :, :],
                            )
                    elif dq % 4 == 3:
                        dqq = dq // 4
                        for b in range(B):
                            for di in range(4):
                                dd0 = 16 * dqq + di
                                nc.sync.dma_start(
                                    out=out[b, g, dd0 : dd0 + 13 : 4, :, :].rearrange(
                                        "dd (hq hr) w -> hq dd hr w", hq=HQ
                                    ),
                                    in_=oq[32 * di : 32 * di + HQ, :, b, :, :],
                                )
    nc.compile()
    return nc


def _make_wm():
    wm = np.zeros((128, 32), np.float16)
    for c in range(CPG):
        for hq in range(HQ):
            wm[c * HQ + hq, hq] = 1.0 / CPG
            wm[c * HQ + hq, 16 + hq] = 1.0 / CPG
    return wm


def _run(left_feature, right_feature, trace=False):
    from concourse.bass_utils import run_bass_kernel_spmd

    if "nc" not in _cache:
        _cache["nc"] = _build_program()
    nc = _cache["nc"]

    left_feature = np.ascontiguousarray(np.asarray(left_feature, dtype=np.float32))
    right_feature = np.ascontiguousarray(np.asarray(right_feature, dtype=np.float32))
    wm = _make_wm()

    in_maps = []
    for i in range(N_CORES):
        c0 = i * CC
        lf = np.ascontiguousarray(left_feature[:, c0 : c0 + CC]).reshape(
            B, G_PER, 128, FD
        )
        rf = np.ascontiguousarray(right_feature[:, c0 : c0 + CC]).reshape(
            B, G_PER, 128, FD
        )
        in_maps.append({"left": lf, "right": rf, "wm": wm})
    res = run_bass_kernel_spmd(nc, in_maps, list(range(N_CORES)), trace=trace)
    shards = [res.results[i]["out"] for i in range(N_CORES)]
    full = np.concatenate([np.asarray(s) for s in shards], axis=1).astype(np.float32)
    # zero-fill the masked triangle (w < d), which the kernel never computes
    for d in range(1, MAX_DISP):
        full[:, :, d, :, :d] = 0.0
    return full, res


def kernel(left_feature, right_feature):
    full, _ = _run(left_feature, right_feature, trace=False)
    return full


# revision 79
# speedup vs baseline: 1.0054x; 1.0026x over previous
"""GwcVolume (group-wise correlation cost volume) Trainium2 Bass kernel.

Problem: left/right features (2, 320, 96, 192) fp32. For each disparity
d in [0, 48): cost[b,g,d,h,w] = mean_c( L[b, g*8+c, h, w] * R[b, g*8+c, h, w-d] )
masked to 0 for w < d.  Output (2, 40, 48, 96, 192) fp32.

Sharding: 40 groups split across 8 cores (5 groups = 40 channels per core).
Per-core inputs slice cleanly along the channel dim; no inter-core comms.

Per-core algorithm (v3):
  - SBUF layout: partitions = (c 8, hq 16), free = (b 2, hr 6, w 192);
    h = hq*6 + hr.  Inputs cast fp32 -> fp16 once (ScalarE), prefetched one
    group ahead.  R stored with a 48-elem zero guard before each w-row.
  - Products in fp16, sliced to w >= w0 = 4*(d//4) (the masked w < d region
    is host-zero-filled).  Per disparity quad: one fused VectorE tensor_mul
    covers di 0..2 via a raw 4-dim AP (stride-0 broadcast of L over the
    di axis, stride -1 shift of R); the di=3 product is split GpSimd
    (hr 0:5) / VectorE (hr 5:6).  The 38:10 row ratio matches the engines'
    model rates, so DVE / GpSimd / PE all run ~88% busy.
  - Group-mean on TensorE: constant block-identity weights [128, 32]
    (wm[(c,hq), s*16+hq'] = 1/8 * delta[hq,hq']), col-tiled 4-wide
    (tile_position=(0, 32*di)) so 4 disparities share one PSUM tile.
    One matmul per (d, hr); PSUM rows padded to 256 f32 so no output
    crosses a 2KB PSUM bank.
  - ScalarE copies PSUM -> SBUF casting to fp16 into a 4-dq-wide staging
    tile; one DMA per (g, dq//4, di, b) ships 4 disparities (step-4 slice
    of the d axis).  The first/last quads are split finer (per-b, per
    hr-pair) to shorten pipeline fill/drain.  Host upcasts to fp32 and
    zero-fills the masked triangle.
"""

import numpy as np

B = 2
C = 320
H = 96
W = 192
GROUP = 40
MAX_DISP = 48
N_CORES = 8
G_PER = GROUP // N_CORES      # 5 groups per core
CPG = C // GROUP              # 8 channels per group
CC = G_PER * CPG              # 40 channels per core
HQ = 16                       # h = hq*HR + hr
HR = 6
FD = HR * W                   # 1152 free elements per partition
GUARD = 8
PITCH = 256                   # psum row pitch (f32): rows never cross banks

# GpSimd takes hr 0:POOL_HR of the di=3 product in every quad; VectorE takes
# the rest.  5/6 makes per-quad times match: DVE (36+2)*0.521 ~ Pool 10*1.983
# ~ PE 48*0.417 (in units of (192-w0) elements).
POOL_HR = 5

_cache = {}


def _w0(g, dq):
    # skip the masked w < 4*dq region everywhere: engines only ever read
    # regions written at the same offset, and the DMA may ship stale or
    # never-written bytes there -- the host zero-fills the whole triangle
    return 4 * dq


def _build_program():
    import concourse.bacc as bacc
    import concourse.tile as tile
    from concourse import mybir

    f32 = mybir.dt.float32
    f16 = mybir.dt.float16

    from concourse.ap import AP

    nc = bacc.Bacc("TRN2", target_bir_lowering=False, num_devices=N_CORES)
    # per-(b,g) channel block (8 ch x 96 x 192) is contiguous = [128, 1152]
    # with partitions=(c, hq), free=(hr, w); declare pre-reshaped for 2D DMAs
    left = nc.declare_dram_parameter("left", [B, G_PER, 128, FD], f32, isOutput=False)
    right = nc.declare_dram_parameter("right", [B, G_PER, 128, FD], f32, isOutput=False)
    wm = nc.declare_dram_parameter("wm", [128, 32], f16, isOutput=False)
    out = nc.declare_dram_parameter(
        "out", [B, G_PER, MAX_DISP, H, W], f16, isOutput=True
    )

    with tile.TileContext(nc) as tc:
        with (
            tc.tile_pool(name="singles", bufs=1) as singles,
            tc.tile_pool(name="stage", bufs=10) as stagep,
            tc.tile_pool(name="res", bufs=1) as res,
            tc.tile_pool(name="prod", bufs=4) as prodp,
            tc.tile_pool(name="oq", bufs=2) as oqp,
            tc.tile_pool(name="psum", bufs=2, space="PSUM") as psump,
        ):
            wm_s = singles.tile([128, 32], f16)
            nc.gpsimd.dma_start(out=wm_s[:, :], in_=wm[:, :])

            Lt, Rt = {}, {}
            for g in range(G_PER):
                Lg = res.tile([128, B, HR, W], f16, tag=f"L{g}")
                Rg = res.tile([128, B, HR, GUARD + W], f16, tag=f"R{g}")
                # only the last 3 guard columns are ever read (products
                # at w in [w0, d) look back at most d - w0 <= 3 elements, and
                # the host zero-fills the masked output region anyway)
                nc.gpsimd.memset(Rg[:, :, :, GUARD - 3 : GUARD], 0.0)
                Lt[g], Rt[g] = Lg, Rg

            def load_one(g, b, which):
                st = stagep.tile([128, FD], f32, tag="stage")
                src = left if which == 0 else right
                if not (g == 0 and b == 0 and which == 1):
                    nc.sync.dma_start(out=st[:, :], in_=src[b, g, :, :])
                if which == 0:
                    nc.scalar.copy(
                        out=Lt[g][:, b, :, :],
                        in_=st[:, :].rearrange("p (hr w) -> p hr w", w=W),
                    )
                elif g == 0 and b == 0:
                    # the first R load+cast is the critical startup chain:
                    # split the DMA and the cast so products start sooner
                    nc.sync.dma_start(
                        out=st[:, 0 : 2 * W], in_=src[b, g, :, 0 : 2 * W]
                    )
                    nc.sync.dma_start(
                        out=st[:, 2 * W : FD], in_=src[b, g, :, 2 * W : FD]
                    )
                    for h0, h1 in ((0, 2), (2, 4), (4, HR)):
                        nc.scalar.copy(
                            out=Rt[g][:, b, h0:h1, GUARD : GUARD + W],
                            in_=st[:, :].rearrange("p (hr w) -> p hr w", w=W)[
                                :, h0:h1, :
                            ],
                        )
                else:
                    nc.scalar.copy(
                        out=Rt[g][:, b, :, GUARD : GUARD + W],
                        in_=st[:, :].rearrange("p (hr w) -> p hr w", w=W),
                    )

            for b in range(B):
                for which in range(2):
                    load_one(0, b, which)

            def fused_mult(P, Lg, Rg, nd, d0, w0, b=None, h0=0, h1=HR):
                # one DVE op for disparities d0..d0+nd-1: di-dim via raw AP
                # (stride 0 broadcast on L, stride -1 shift on R); (b,hr)
                # merge into one stride-192/240 dim to stay within 4 AP dims
                X = W - w0
                if b is None:
                    pv = P[:, 0, 0, 0, w0:W]
                    lv = Lg[:, 0, 0, w0:W]
                    bh = [4 * HR, B * HR]
                else:
                    pv = P[:, 0, b, h0, w0:W]
                    lv = Lg[:, b, h0, w0:W]
                    bh = [4 * HR, h1 - h0]
                out_ap = AP(
                    pv.tensor, pv.offset,
                    [list(pv.ap[0]), [B * HR * W, nd], [W, bh[1]], [1, X]],
                )
                l_ap = AP(
                    lv.tensor, lv.offset,
                    [list(lv.ap[0]), [0, nd], [W, bh[1]], [1, X]],
                )
                rv = Rg[:, 0 if b is None else b, h0 if b is not None else 0, 0:1]
                r_ap = AP(
                    rv.tensor, rv.offset + GUARD + w0 - d0,
                    [list(rv.ap[0]), [-1, nd], [GUARD + W, bh[1]], [1, X]],
                )
                nc.vector.tensor_mul(out_ap, l_ap, r_ap)

            for g in range(G_PER):
                Lg, Rg = Lt[g], Rt[g]
                oq = None
                for dq in range(MAX_DISP // 4):
                    w0 = _w0(g, dq)
                    d0 = 4 * dq
                    if dq % 4 == 0:
                        oq = oqp.tile([128, 4, B, HR, W], f16, tag="oq")
                    P = prodp.tile([128, 4, B, HR, W], f16, tag="P")
                    d3 = d0 + 3
                    if (g == 0 and dq < 2) or (
                        g == G_PER - 1 and dq in (0, MAX_DISP // 4 - 1)
                    ):
                        # split per-b at the ends of the pipeline (and per
                        # hr-pair for the first and last quads) so the first
                        # matmuls start sooner / the last ones finish sooner
                        lastq = g == G_PER - 1 and dq == MAX_DISP // 4 - 1
                        for b in range(B):
                            if (g == 0 and dq == 0) or lastq:
                                for h0 in range(0, HR, 2):
                                    fused_mult(
                                        P, Lg, Rg, 3, d0, w0, b=b, h0=h0, h1=h0 + 2
                                    )
                            else:
                                fused_mult(P, Lg, Rg, 3, d0, w0, b=b)
                            if lastq:
                                nc.vector.tensor_mul(
                                    P[:, 3, b, :, w0:W],
                                    Lg[:, b, :, w0:W],
                                    Rg[:, b, :, GUARD + w0 - d3 : GUARD + W - d3],
                                )
                                continue
                            nc.gpsimd.tensor_mul(
                                P[:, 3, b, 0:POOL_HR, w0:W],
                                Lg[:, b, 0:POOL_HR, w0:W],
                                Rg[:, b, 0:POOL_HR, GUARD + w0 - d3 : GUARD + W - d3],
                            )
                            nc.vector.tensor_mul(
                                P[:, 3, b, POOL_HR:HR, w0:W],
                                Lg[:, b, POOL_HR:HR, w0:W],
                                Rg[:, b, POOL_HR:HR, GUARD + w0 - d3 : GUARD + W - d3],
                            )
                    else:
                        fused_mult(P, Lg, Rg, 3, d0, w0)
                        nc.gpsimd.tensor_mul(
                            P[:, 3, :, 0:POOL_HR, w0:W],
                            Lg[:, :, 0:POOL_HR, w0:W],
                            Rg[:, :, 0:POOL_HR, GUARD + w0 - d3 : GUARD + W - d3],
                        )
                        nc.vector.tensor_mul(
                            P[:, 3, :, POOL_HR:HR, w0:W],
                            Lg[:, :, POOL_HR:HR, w0:W],
                            Rg[:, :, POOL_HR:HR, GUARD + w0 - d3 : GUARD + W - d3],
                        )
                    # prefetch two groups ahead so the late group
                    # boundaries carry no cast traffic at all
                    if g == 0 and dq in (4, 7):
                        b = 0 if dq == 4 else 1
                        load_one(1, b, 0)
                        load_one(1, b, 1)
                    elif g == 0 and dq in (9, 11):
                        b = 0 if dq == 9 else 1
                        load_one(2, b, 0)
                        load_one(2, b, 1)
                    elif 1 <= g <= 2 and dq in (4, 7):
                        b = 0 if dq == 4 else 1
                        load_one(g + 2, b, 0)
                        load_one(g + 2, b, 1)
                    for b in range(B):
                        pq = psump.tile([128, HR, PITCH], f32, tag="pq")
                        for hr in range(HR):
                            for di in range(4):
                                nc.tensor.matmul(
                                    pq[32 * di : 32 * di + 32, hr, w0:W],
                                    wm_s[:, :],
                                    P[:, di, b, hr, w0:W],
                                    start=True,
                                    stop=True,
                                    tile_position=(0, 32 * di),
                                )
                        if g == G_PER - 1 and dq == MAX_DISP // 4 - 1:
                            # very last copy: split across DVE (idle by now)
                            # and ScalarE to shorten the drain tail
                            nc.vector.tensor_scalar_mul(
                                oq[:, dq % 4, b, :, w0:W], pq[:, :, w0:W], 1.0
                            )
                        else:
                            nc.scalar.copy(
                                out=oq[:, dq % 4, b, :, w0:W],
                                in_=pq[:, :, w0:W],
                            )
                    last_oq = g == G_PER - 1 and dq // 4 == 2
                    if last_oq:
                        # ship each dq as soon as both its copies are done
                        # (b merged; only dq 11's four DMAs remain in the tail)
                        last2 = g == G_PER - 1 and dq == MAX_DISP // 4 - 1
                        for di in range(4):
                            d = 4 * dq + di
                            # route half the very last DMAs via GpSimd SWDGE
                            # (bypasses the shared HWDGE issuer; Pool is idle)
                            eng = nc.gpsimd if last2 and di >= 3 else nc.sync
                            eng.dma_start(
                                out=out[:, g, d, :, :].rearrange(
                                    "b (hq hr) w -> hq b hr w", hq=HQ
                                ),
                                in_=oq[32 * di : 32 * di + HQ, dq % 4, :, :, :],
                            )
                    elif dq % 4 == 3:
                        dqq = dq // 4
                        for b in range(B):
                            for di in range(4):
                                dd0 = 16 * dqq + di
                                nc.sync.dma_start(
                                    out=out[b, g, dd0 : dd0 + 13 : 4, :, :].rearrange(
                                        "dd (hq hr) w -> hq dd hr w", hq=HQ
                                    ),
                                    in_=oq[32 * di : 32 * di + HQ, :, b, :, :],
                                )
    nc.compile()
    return nc


def _make_wm():
    wm = np.zeros((128, 32), np.float16)
    for c in range(CPG):
        for hq in range(HQ):
            wm[c * HQ + hq, hq] = 1.0 / CPG
            wm[c * HQ + hq, 16 + hq] = 1.0 / CPG
    return wm


def _run(left_feature, right_feature, trace=False):
    from concourse.bass_utils import run_bass_kernel_spmd

    if "nc" not in _cache:
        _cache["nc"] = _build_program()
    nc = _cache["nc"]

    left_feature = np.ascontiguousarray(np.asarray(left_feature, dtype=np.float32))
    right_feature = np.ascontiguousarray(np.asarray(right_feature, dtype=np.float32))
    wm = _make_wm()

    in_maps = []
    for i in range(N_CORES):
        c0 = i * CC
        lf = np.ascontiguousarray(left_feature[:, c0 : c0 + CC]).reshape(
            B, G_PER, 128, FD
        )
        rf = np.ascontiguousarray(right_feature[:, c0 : c0 + CC]).reshape(
            B, G_PER, 128, FD
        )
        in_maps.append({"left": lf, "right": rf, "wm": wm})
    res = run_bass_kernel_spmd(nc, in_maps, list(range(N_CORES)), trace=trace)
    shards = [res.results[i]["out"] for i in range(N_CORES)]
    full = np.concatenate([np.asarray(s) for s in shards], axis=1).astype(np.float32)
    # zero-fill the masked triangle (w < d), which the kernel never computes
    for d in range(1, MAX_DISP):
        full[:, :, d, :, :d] = 0.0
    return full, res


def kernel(left_feature, right_feature):
    full, _ = _run(left_feature, right_feature, trace=False)
    return full
